# revision 36
# baseline (speedup 1.0000x reference)
"""Causal self-attention on 8 trn2 NeuronCores.

Sharding: core = 2*b + g  (b = batch 0..3, g = head-group 0..1).
Each core computes 8 heads (feature slice of 512) for one batch element and
produces a partial output projection; host sums the two partials per batch
and adds bp.

Device layout (per core):
  phase 1: qT/kT produced feature-major ([feat, tok], i.e. transposed) and
           v token-major, directly from matmuls against host-pre-transposed
           x and weight slices -- no on-device transposes anywhere.
  phase 2: scores are computed transposed, sT[k, q] = kT_tile.T @ qT_tile,
           two heads packed per 128-partition tile and row-tiled through the
           PE concurrently. exp on ScalarE (PSUM->SBUF, bf16), causal mask
           applied only to the 128x128 diagonal triangles. AV matmul uses v
           augmented with a ones column so each head's softmax denominator
           falls out of the same accumulation for free.
  phase 3: yT is already [feat, tok] => output projection directly.
"""

import os

import numpy as np
import ml_dtypes

import concourse.bass as bass
import concourse.mybir as mybir
import concourse.tile as tile
from concourse import bacc
from concourse.bass_utils import run_bass_kernel_spmd

B, T, C, H, D = 4, 2048, 1024, 16, 64
NCORES = 8
G = 2              # head groups
F = C // G         # 512 features per core
HL = H // G        # 8 local heads
NP = HL // 2       # 4 head pairs (2 heads / 128-partition tile)
TCH = 512          # token chunk (free dim of most matmuls)
NTC = T // TCH     # 4
KBLK = 128         # k block inside attention
f32 = mybir.dt.float32
f32r = mybir.dt.float32r
bf16 = mybir.dt.bfloat16

LAST_EXEC_TIME_NS = None
_CACHE = {}


def _bcast_part(ap, n):
    """AP reading the same (single-partition) row n times: partition step 0."""
    a = list(ap.ap)
    a[0] = [0, n]
    return bass.AP(tensor=ap.tensor, offset=ap.offset, ap=a)


def _emit(nc, tc, io, dbg=None):
    xT, wq, wk, wv, wp, bqs, bks, bvr, tri_d, ones_d, vones_d, out = io
    ctxpools = []

    def pool(name, bufs, space="SBUF"):
        p = tc.alloc_tile_pool(name=name, bufs=bufs, space=space)
        ctxpools.append(p)
        return p

    consts = pool("consts", 1)
    xcp = pool("xcp", 16)
    wqp = pool("wqp", 8)
    wkp = pool("wkp", 8)
    wvp = pool("wvp", 8)
    wpp = pool("wpp", 4)
    qcp = pool("qcp", 8)
    kcp = pool("kcp", 16)
    vp = pool("vp", 16)
    ep = pool("ep", 4)
    ytp = pool("ytp", 6)
    ostp = pool("ostp", 3)
    dbp = pool("dbp", 3)
    rp = pool("rp", 3)
    drp = pool("drp", 4, space="DRAM")
    ps_mm = pool("ps_mm", 2, space="PSUM")
    ps_s = pool("ps_s", 2, space="PSUM")
    ps_y = pool("ps_y", 2, space="PSUM")

    # ---- constants (the big ones are loaded inside load_wkv, after the
    # startup-critical x and wq transfers) ----
    tri = consts.tile([128, 128], bf16)
    bvb = consts.tile([128, TCH], f32)
    bqt = consts.tile([128, NP], f32)
    bkt = consts.tile([128, NP], f32)

    # ---- resident weights (wq first so chunk-0 q-proj starts ASAP; wp
    # deferred to a filler task since it's not needed until out-proj) ----
    wq_sb, wk_sb, wv_sb, wp_sb = [], [], [], []


    def load_wkv():
        nc.sync.dma_start(out=bqt, in_=bqs)
        nc.sync.dma_start(out=bkt, in_=bks)
        nc.sync.dma_start(out=tri, in_=tri_d)
        nc.sync.dma_start(out=bvb, in_=_bcast_part(bvr, 128))
        for kc in range(8):
            wkt = wkp.tile([128, F], bf16, name=f"wk{kc}", tag="wk")
            nc.sync.dma_start(out=wkt, in_=wk[kc * 128:(kc + 1) * 128, :])
            wk_sb.append(wkt)
            wvt = wvp.tile([128, F], bf16, name=f"wv{kc}", tag="wv")
            nc.sync.dma_start(out=wvt, in_=wv[kc * 128:(kc + 1) * 128, :])
            wv_sb.append(wvt)

    def load_wp():
        for fc in range(4):
            wpt = wpp.tile([128, C], bf16, name=f"wp{fc}", tag="wp")
            nc.sync.dma_start(out=wpt, in_=wp[fc * 128:(fc + 1) * 128, :])
            wp_sb.append(wpt)

    k_sb = {}     # (pair, chunk) -> [128, 512] bf16 kT chunk
    v_sb = {}     # token-tile -> [128, 8, 65] bf16 v_aug
    q_sb = {}     # (pair, chunk) -> [128, 512] bf16 qT chunk

    def proj_gen(tcj, with_wq=False):
        """Generator of small emission steps (~1-2 matmuls each) for token
        chunk tcj's projections, so the attention loop of the previous
        chunk can drip-feed them between its own matmul bursts. with_wq
        interleaves the wq loads so the first chain's operands (x0, wq0)
        are at the head of the DMA queues."""
        xc = []
        for kc in range(8):
            xt = xcp.tile([128, TCH], bf16, name=f"x{tcj}_{kc}", tag="xc")
            nc.sync.dma_start(out=xt, in_=xT[tcj, kc])
            xc.append(xt)
            if with_wq:
                wqt = wqp.tile([128, F], bf16, name=f"wq{kc}", tag="wq")
                nc.sync.dma_start(out=wqt, in_=wq[kc * 128:(kc + 1) * 128, :])
                wq_sb.append(wqt)
        yield
        for p in range(NP):
            psq = ps_mm.tile([128, TCH], f32, name=f"psq{tcj}_{p}", tag="mm")
            for kc in range(8):
                nc.tensor.matmul(
                    psq, wq_sb[kc][:, p * 128:(p + 1) * 128], xc[kc],
                    start=(kc == 0), stop=(kc == 7))
                if kc % 2 == 1:
                    yield
            qt = qcp.tile([128, TCH], bf16, name=f"q{tcj}_{p}", tag="qc")
            nc.vector.tensor_scalar_add(qt, psq, bqt[:, p:p + 1])
            if dbg is not None and tcj == 0 and p == 0:
                nc.sync.dma_start(out=dbg["q"], in_=qt)
            q_sb[(p, tcj)] = qt
            yield
            psk = ps_mm.tile([128, TCH], f32, name=f"psk{tcj}_{p}", tag="mm")
            for kc in range(8):
                nc.tensor.matmul(
                    psk, wk_sb[kc][:, p * 128:(p + 1) * 128], xc[kc],
                    start=(kc == 0), stop=(kc == 7))
                if kc % 2 == 1:
                    yield
            kt = kcp.tile([128, TCH], bf16, name=f"k{tcj}_{p}", tag="kc")
            nc.vector.tensor_scalar_add(kt, psk, bkt[:, p:p + 1])
            if dbg is not None and tcj == 0 and p == 0:
                nc.sync.dma_start(out=dbg["k"], in_=kt)
            k_sb[(p, tcj)] = kt
            yield
        for tt in range(4):
            ti = 4 * tcj + tt
            psv = ps_mm.tile([128, TCH], f32, name=f"psv{ti}", tag="mm")
            for kc in range(8):
                nc.tensor.matmul(
                    psv, xc[kc][:, tt * 128:(tt + 1) * 128], wv_sb[kc],
                    start=(kc == 0), stop=(kc == 7))
                if kc % 2 == 1:
                    yield
            vt = vp.tile([128, HL, D + 1], bf16, name=f"v{ti}", tag="v")
            nc.vector.tensor_add(
                vt[:, :, 0:D],
                psv.rearrange("p (h d) -> p h d", h=HL),
                bvb.rearrange("p (h d) -> p h d", h=HL))
            nc.vector.memset(vt[:, :, D:D + 1], 1.0)
            if dbg is not None and ti == 0:
                nc.sync.dma_start(out=dbg["v"], in_=vt)
            v_sb[ti] = vt
            yield

    def outproj_gen(j, yts):
        # the two co-chains of a token tile are emitted as a wave: both
        # run their fc0-2 matmuls before either fc3, so a late yts[3]
        # (last pair's normalize) doesn't block independent work in the
        # static PE stream
        for tt in range(4):
            row0 = j * TCH + tt * 128
            pso2 = []
            for co in range(2):
                pso = ps_mm.tile([128, TCH], f32,
                                 name=f"po{j}_{tt}_{co}", tag="mm")
                pso2.append(pso)
                for fc in range(3):
                    nc.tensor.matmul(
                        pso, yts[fc][:, tt * 128:(tt + 1) * 128],
                        wp_sb[fc][:, co * TCH:(co + 1) * TCH],
                        start=(fc == 0), stop=False)
                    if fc % 2 == 1:
                        yield
            for co in range(2):
                nc.tensor.matmul(
                    pso2[co], yts[3][:, tt * 128:(tt + 1) * 128],
                    wp_sb[3][:, co * TCH:(co + 1) * TCH],
                    start=False, stop=True)
                ost = ostp.tile([128, TCH], f32,
                                name=f"o{j}_{tt}_{co}", tag="o")
                nc.vector.tensor_copy(ost, pso2[co])
                nc.sync.dma_start(
                    out=out[row0:row0 + 128, co * TCH:(co + 1) * TCH],
                    in_=ost)
                yield

    def attention(j, fillq):
        def fill(n):
            while n > 0 and fillq:
                try:
                    next(fillq[0][2])
                    n -= 1
                except StopIteration:
                    fillq.pop(0)

        nkb = 4 * (j + 1)
        yts = []
        for p in range(NP):
            psyA = ps_y.tile([D + 1, TCH], f32, name=f"yA{j}_{p}", tag="y")
            psyB = ps_y.tile([D + 1, TCH], f32, name=f"yB{j}_{p}", tag="y")
            qA = q_sb[(p, j)][0:64, :]
            qB = q_sb[(p, j)][64:128, :]

            def emit_scores(kb):
                tck, wb = kb // 4, kb % 4
                kcc = k_sb[(p, tck)]
                pss = ps_s.tile([128, 2 * TCH], f32,
                                name=f"s{j}_{p}_{kb}", tag="s")
                so = max(0, 128 * (kb - 4 * j))  # valid q starts here
                nc.tensor.matmul(pss[:, so:TCH],
                                 kcc[0:64, wb * 128:(wb + 1) * 128],
                                 qA[:, so:TCH], start=True, stop=True)
                nc.tensor.matmul(pss[:, TCH + so:2 * TCH],
                                 kcc[64:128, wb * 128:(wb + 1) * 128],
                                 qB[:, so:TCH], start=True, stop=True)
                ee = ep.tile([128, 2 * TCH], bf16, name=f"e{j}_{p}_{kb}",
                             tag="e")
                di = kb - 4 * j
                if di <= 0:
                    nc.scalar.activation(ee, pss,
                                         mybir.ActivationFunctionType.Exp)
                else:
                    qo = 128 * di
                    evo = bass.AP(tensor=ee.tensor, offset=ee.offset + qo,
                                  ap=[ee.ap[0], [TCH, 2], [1, TCH - qo]])
                    evi = bass.AP(tensor=pss.tensor, offset=pss.offset + qo,
                                  ap=[pss.ap[0], [TCH, 2], [1, TCH - qo]])
                    nc.scalar.activation(evo, evi,
                                         mybir.ActivationFunctionType.Exp)
                if di >= 0:
                    qoff = 128 * di
                    evs = bass.AP(tensor=ee.tensor,
                                  offset=ee.offset + qoff,
                                  ap=[ee.ap[0], [TCH, 2], [1, 128]])
                    triv = bass.AP(tensor=tri.tensor, offset=tri.offset,
                                   ap=[tri.ap[0], [0, 2], [1, 128]])
                    nc.vector.tensor_mul(evs, evs, triv)
                if dbg is not None and j == 0 and p == 0 and kb < 2:
                    nc.sync.dma_start(out=dbg["e"][:, kb * TCH:(kb + 1) * TCH],
                                      in_=ee[:, 0:TCH])
                return ee

            def emit_av(kb, ee):
                di = kb - 4 * j
                qoff = 0 if di < 0 else 128 * di
                vt = v_sb[kb]
                nc.tensor.matmul(
                    psyA[:, qoff:TCH], vt[:, 2 * p, :],
                    ee[:, qoff:TCH],
                    start=(kb == 0), stop=(kb == nkb - 1))
                nc.tensor.matmul(
                    psyB[:, qoff:TCH], vt[:, 2 * p + 1, :],
                    ee[:, TCH + qoff:2 * TCH],
                    start=(kb == 0), stop=(kb == nkb - 1))

            pend = []
            for kb in range(nkb):
                ee = emit_scores(kb)
                if len(pend) >= 2:
                    emit_av(*pend.pop(0))
                fill(2)
                pend.append((kb, ee))
            for pr in pend:
                emit_av(*pr)

            yt = ytp.tile([128, TCH], bf16, name=f"yt{j}_{p}", tag="yt")
            for hh, psy in enumerate((psyA, psyB)):
                # free the PSUM bank fast: one whole-tile copy to SBUF, then
                # the whole normalize chain (DRAM-broadcast the denom row,
                # reciprocal, multiply) runs off SBUF with no PE and no
                # PSUM-slot involvement.
                ycp = rp.tile([D + 1, TCH], f32, name=f"yc{j}_{p}_{hh}",
                              tag="yc")
                nc.vector.tensor_copy(ycp, psy)
                dsc = drp.tile([1, TCH], f32, name=f"ds{j}_{p}_{hh}",
                               tag="dscr")
                nc.sync.dma_start(out=dsc, in_=ycp[D:D + 1, :])
                dbr = dbp.tile([64, TCH], f32, name=f"dbr{j}_{p}_{hh}",
                               tag="dbr")
                nc.sync.dma_start(out=dbr, in_=_bcast_part(dsc, 64))
                dbs = dbp.tile([64, TCH], f32, name=f"dbs{j}_{p}_{hh}",
                               tag="db")
                nc.vector.reciprocal_approx_fast(dbs, dbr)
                if dbg is not None and j == 0 and p == 0 and hh == 0:
                    nc.sync.dma_start(out=dbg["y"], in_=ycp)
                    nc.sync.dma_start(out=dbg["r"], in_=ycp[D:D + 1, :])
                    nc.sync.dma_start(out=dbg["db"], in_=dbs)
                if hh == 0:
                    nc.vector.tensor_mul(yt[0:64, :], ycp[0:D, :], dbs)
                else:
                    ytb = dbp.tile([64, TCH], bf16, name=f"ytb{j}_{p}",
                                   tag="ytb")
                    nc.vector.tensor_mul(ytb, ycp[0:D, :], dbs)
                    nc.sync.dma_start(out=yt[64:128, :], in_=ytb)
            if dbg is not None and j == 0 and p == 0:
                nc.sync.dma_start(out=dbg["yt"], in_=yt)
            yts.append(yt)
            fill(3)
        return yts

    # chunk 0 projections up front (q first while wk/wv still loading),
    # then attention(j) drip-feeds projections of chunk j+1 and the
    # output projection of chunk j-1 between its matmul bursts
    g0 = proj_gen(0, with_wq=True)
    next(g0)          # interleaved x + wq loads go first
    load_wkv()
    for _ in g0:
        pass
    load_wp()
    fillq = []   # entries: (kind, idx, generator)
    for j in range(NTC):
        if j + 1 < NTC:
            fillq.append(("proj", j + 1, proj_gen(j + 1)))
        # attention(j) requires proj(j) fully emitted: force-drain overdue
        for ent in [e for e in fillq if e[0] == "proj" and e[1] <= j]:
            for _ in ent[2]:
                pass
            fillq.remove(ent)
        yts = attention(j, fillq)
        fillq.append(("out", j, outproj_gen(j, yts)))
    for _, _, g in fillq:
        for _ in g:
            pass

    for p in reversed(ctxpools):
        p.release()


def _build(debug=False):
    key = ("nc", debug)
    if key in _CACHE:
        return _CACHE[key]
    nc = bacc.Bacc("TRN2", target_bir_lowering=False, debug=False)
    xT = nc.dram_tensor("xT", [NTC, 8, 128, TCH], bf16,
                    kind="ExternalInput").ap()
    wq = nc.dram_tensor("wqT", [C, F], bf16, kind="ExternalInput").ap()
    wk = nc.dram_tensor("wkT", [C, F], bf16, kind="ExternalInput").ap()
    wv = nc.dram_tensor("wvT", [C, F], bf16, kind="ExternalInput").ap()
    wp = nc.dram_tensor("wpT", [F, C], bf16, kind="ExternalInput").ap()
    bqs = nc.dram_tensor("bqs", [128, NP], f32, kind="ExternalInput").ap()
    bks = nc.dram_tensor("bks", [128, NP], f32, kind="ExternalInput").ap()
    bvr = nc.dram_tensor("bvr", [1, F], f32, kind="ExternalInput").ap()
    tri = nc.dram_tensor("tri", [128, 128], bf16, kind="ExternalInput").ap()
    out = nc.dram_tensor("out", [T, C], f32, kind="ExternalOutput").ap()
    dbg = None
    if debug:
        dbg = {
            "q": nc.dram_tensor("dbg_q", [128, TCH], bf16,
                                kind="ExternalOutput").ap(),
            "k": nc.dram_tensor("dbg_k", [128, TCH], bf16,
                                kind="ExternalOutput").ap(),
            "v": nc.dram_tensor("dbg_v", [128, HL, D + 1], bf16,
                                kind="ExternalOutput").ap(),
            "e": nc.dram_tensor("dbg_e", [128, 2 * TCH], bf16,
                                kind="ExternalOutput").ap(),
            "y": nc.dram_tensor("dbg_y", [D + 1, TCH], f32,
                                kind="ExternalOutput").ap(),
            "yt": nc.dram_tensor("dbg_yt", [128, TCH], bf16,
                                 kind="ExternalOutput").ap(),
            "r": nc.dram_tensor("dbg_r", [1, TCH], f32,
                                kind="ExternalOutput").ap(),
            "db": nc.dram_tensor("dbg_db", [64, TCH], f32,
                                 kind="ExternalOutput").ap(),
        }
    with tile.TileContext(nc) as tc:
        _emit(nc, tc, (xT, wq, wk, wv, wp, bqs, bks, bvr, tri, None, None,
                       out), dbg=dbg)
    nc.compile()
    _CACHE[key] = nc
    return nc


def make_in_maps(x, Wq, bq, Wk, bk, Wv, bv, Wp, bp):
    tri = np.triu(np.ones((128, 128), np.float32)).astype(ml_dtypes.bfloat16)
    in_maps = []
    for core in range(NCORES):
        b, g = core // G, core % G
        sl = slice(g * F, (g + 1) * F)
        in_maps.append({
            "xT": np.ascontiguousarray(
                x[b].T.reshape(8, 128, NTC, TCH).transpose(2, 0, 1, 3)
            ).astype(ml_dtypes.bfloat16),
            "wqT": np.ascontiguousarray(
                (Wq[sl, :] * 0.125).T).astype(ml_dtypes.bfloat16),
            "wkT": np.ascontiguousarray(Wk[sl, :].T).astype(
                ml_dtypes.bfloat16),
            "wvT": np.ascontiguousarray(Wv[sl, :].T).astype(
                ml_dtypes.bfloat16),
            "wpT": np.ascontiguousarray(Wp[:, sl].T).astype(
                ml_dtypes.bfloat16),
            "bqs": np.ascontiguousarray((bq[sl] * 0.125).reshape(NP, 128).T),
            "bks": np.ascontiguousarray(bk[sl].reshape(NP, 128).T),
            "bvr": bv[sl].reshape(1, F).copy(),
            "tri": tri,
        })
    return in_maps


def _maybe_install_trace_shim():
    """Enable NTFF profiling under axon when BASS_TRACE is set."""
    import sys
    import types
    if "antenv.axon_hooks" in sys.modules:
        return
    try:
        from trn_agent_boot.trn_boot import _ntff_profile_via_ctypes
        hook = _ntff_profile_via_ctypes("/opt/axon/libaxon_pjrt.so")
    except Exception:
        return
    mod = types.ModuleType("antenv.axon_hooks")
    mod.get_axon_ntff_profile_hook = lambda: hook
    mod.set_axon_ntff_profile_hook = lambda h: None
    sys.modules["antenv.axon_hooks"] = mod


def _run_device(x, Wq, bq, Wk, bk, Wv, bv, Wp, bp):
    global LAST_EXEC_TIME_NS
    trace = os.environ.get("BASS_TRACE", "") not in ("", "0")
    if trace:
        _maybe_install_trace_shim()
    nc = _build()
    in_maps = make_in_maps(x, Wq, bq, Wk, bk, Wv, bv, Wp, bp)
    res = run_bass_kernel_spmd(nc, in_maps, list(range(NCORES)), trace=trace)
    LAST_EXEC_TIME_NS = res.exec_time_ns
    out = np.empty((B, T, C), np.float32)
    for b in range(B):
        out[b] = res.results[2 * b]["out"] + res.results[2 * b + 1]["out"] + bp
    return out


def kernel(x, Wq, bq, Wk, bk, Wv, bv, Wp, bp):
    """Run on device in an isolated subprocess with retries: a rare
    transient device fault poisons the whole PJRT process, so isolation is
    the only way to recover and rerun."""
    global LAST_EXEC_TIME_NS
    import subprocess
    import sys
    import tempfile
    args = {k: np.asarray(v, np.float32) for k, v in dict(
        x=x, Wq=Wq, bq=bq, Wk=Wk, bk=bk, Wv=Wv, bv=bv, Wp=Wp, bp=bp).items()}
    if os.environ.get("BASS_KERNEL_INPROC"):
        return _run_device(**args)
    last = None
    for attempt in range(4):
        with tempfile.TemporaryDirectory() as td:
            inp = os.path.join(td, "in.npz")
            outp = os.path.join(td, "out.npz")
            np.savez(inp, **args)
            r = subprocess.run(
                [sys.executable, os.path.abspath(__file__),
                 "--worker", inp, outp],
                capture_output=True, text=True, timeout=3000)
            if r.returncode == 0 and os.path.exists(outp):
                with np.load(outp) as z:
                    out = z["out"]
                    t = z["exec_ns"]
                LAST_EXEC_TIME_NS = None if t < 0 else int(t)
                return out
            last = (r.returncode, r.stdout[-2000:], r.stderr[-2000:])
    raise RuntimeError(f"device run failed after retries: {last}")


def _worker(inp, outp):
    with np.load(inp) as z:
        args = {k: z[k] for k in z.files}
    out = _run_device(**args)
    t = -1 if LAST_EXEC_TIME_NS is None else int(LAST_EXEC_TIME_NS)
    np.savez(outp, out=out, exec_ns=t)


if __name__ == "__main__":
    import sys
    if len(sys.argv) == 4 and sys.argv[1] == "--worker":
        _worker(sys.argv[2], sys.argv[3])


# revision 38
# speedup vs baseline: 1.0344x; 1.0344x over previous
"""Causal self-attention on 8 trn2 NeuronCores.

Sharding: core = 2*b + g  (b = batch 0..3, g = head-group 0..1).
Each core computes 8 heads (feature slice of 512) for one batch element and
produces a partial output projection; host sums the two partials per batch
and adds bp.

Device layout (per core):
  phase 1: qT/kT produced feature-major ([feat, tok], i.e. transposed) and
           v token-major, directly from matmuls against host-pre-transposed
           x and weight slices -- no on-device transposes anywhere.
  phase 2: scores are computed transposed, sT[k, q] = kT_tile.T @ qT_tile,
           two heads packed per 128-partition tile and row-tiled through the
           PE concurrently. exp on ScalarE (PSUM->SBUF, bf16), causal mask
           applied only to the 128x128 diagonal triangles. AV matmul uses v
           augmented with a ones column so each head's softmax denominator
           falls out of the same accumulation for free.
  phase 3: yT is already [feat, tok] => output projection directly.
"""

import os

import numpy as np
import ml_dtypes

import concourse.bass as bass
import concourse.mybir as mybir
import concourse.tile as tile
from concourse import bacc
from concourse.bass_utils import run_bass_kernel_spmd

B, T, C, H, D = 4, 2048, 1024, 16, 64
NCORES = 8
G = 2              # head groups
F = C // G         # 512 features per core
HL = H // G        # 8 local heads
NP = HL // 2       # 4 head pairs (2 heads / 128-partition tile)
TCH = 512          # token chunk (free dim of most matmuls)
NTC = T // TCH     # 4
KBLK = 128         # k block inside attention
f32 = mybir.dt.float32
f32r = mybir.dt.float32r
bf16 = mybir.dt.bfloat16

LAST_EXEC_TIME_NS = None
_CACHE = {}


def _bcast_part(ap, n):
    """AP reading the same (single-partition) row n times: partition step 0."""
    a = list(ap.ap)
    a[0] = [0, n]
    return bass.AP(tensor=ap.tensor, offset=ap.offset, ap=a)


def _emit(nc, tc, io, dbg=None):
    xT, wq, wk, wv, wp, bqs, bks, bvr, tri_d, ones_d, vones_d, out = io
    ctxpools = []

    def pool(name, bufs, space="SBUF"):
        p = tc.alloc_tile_pool(name=name, bufs=bufs, space=space)
        ctxpools.append(p)
        return p

    consts = pool("consts", 1)
    xcp = pool("xcp", 16)
    wqp = pool("wqp", 8)
    wkp = pool("wkp", 8)
    wvp = pool("wvp", 8)
    wpp = pool("wpp", 4)
    qcp = pool("qcp", 8)
    kcp = pool("kcp", 16)
    vp = pool("vp", 16)
    ep = pool("ep", 4)
    ytp = pool("ytp", 6)
    ostp = pool("ostp", 3)
    dbp = pool("dbp", 3)
    rp = pool("rp", 3)
    drp = pool("drp", 4, space="DRAM")
    ps_mm = pool("ps_mm", 2, space="PSUM")
    ps_s = pool("ps_s", 2, space="PSUM")
    ps_y = pool("ps_y", 2, space="PSUM")

    # ---- constants (the big ones are loaded inside load_wkv, after the
    # startup-critical x and wq transfers) ----
    tri = consts.tile([128, 128], bf16)
    bvb = consts.tile([128, TCH], f32)
    bqt = consts.tile([128, NP], f32)
    bkt = consts.tile([128, NP], f32)

    # ---- resident weights (wq first so chunk-0 q-proj starts ASAP; wp
    # deferred to a filler task since it's not needed until out-proj) ----
    wq_sb, wk_sb, wv_sb, wp_sb = [], [], [], []


    def load_wkv():
        nc.sync.dma_start(out=bqt, in_=bqs)
        nc.sync.dma_start(out=bkt, in_=bks)
        nc.sync.dma_start(out=tri, in_=tri_d)
        nc.sync.dma_start(out=bvb, in_=_bcast_part(bvr, 128))
        for kc in range(8):
            wkt = wkp.tile([128, F], bf16, name=f"wk{kc}", tag="wk")
            nc.sync.dma_start(out=wkt, in_=wk[kc * 128:(kc + 1) * 128, :])
            wk_sb.append(wkt)
            wvt = wvp.tile([128, F], bf16, name=f"wv{kc}", tag="wv")
            nc.sync.dma_start(out=wvt, in_=wv[kc * 128:(kc + 1) * 128, :])
            wv_sb.append(wvt)

    def load_wp():
        for fc in range(4):
            wpt = wpp.tile([128, C], bf16, name=f"wp{fc}", tag="wp")
            nc.sync.dma_start(out=wpt, in_=wp[fc * 128:(fc + 1) * 128, :])
            wp_sb.append(wpt)

    k_sb = {}     # (pair, chunk) -> [128, 512] bf16 kT chunk
    v_sb = {}     # token-tile -> [128, 8, 65] bf16 v_aug
    q_sb = {}     # (pair, chunk) -> [128, 512] bf16 qT chunk

    def proj_gen(tcj, with_wq=False):
        """Generator of small emission steps (~1-2 matmuls each) for token
        chunk tcj's projections, so the attention loop of the previous
        chunk can drip-feed them between its own matmul bursts. with_wq
        interleaves the wq loads so the first chain's operands (x0, wq0)
        are at the head of the DMA queues."""
        xc = []
        for kc in range(8):
            xt = xcp.tile([128, TCH], bf16, name=f"x{tcj}_{kc}", tag="xc")
            nc.sync.dma_start(out=xt, in_=xT[tcj, kc])
            xc.append(xt)
            if with_wq:
                wqt = wqp.tile([128, F], bf16, name=f"wq{kc}", tag="wq")
                nc.sync.dma_start(out=wqt, in_=wq[kc * 128:(kc + 1) * 128, :])
                wq_sb.append(wqt)
        yield
        for p in range(NP):
            psq = ps_mm.tile([128, TCH], f32, name=f"psq{tcj}_{p}", tag="mm")
            for kc in range(8):
                nc.tensor.matmul(
                    psq, wq_sb[kc][:, p * 128:(p + 1) * 128], xc[kc],
                    start=(kc == 0), stop=(kc == 7))
                if kc % 2 == 1:
                    yield
            qt = qcp.tile([128, TCH], bf16, name=f"q{tcj}_{p}", tag="qc")
            # drain on ScalarE (idle during projections) so the PSUM slot
            # frees without queueing behind DVE's normalize/mask work
            nc.scalar.activation(qt, psq, mybir.ActivationFunctionType.Identity,
                                 bias=bqt[:, p:p + 1])
            if dbg is not None and tcj == 0 and p == 0:
                nc.sync.dma_start(out=dbg["q"], in_=qt)
            q_sb[(p, tcj)] = qt
            yield
            psk = ps_mm.tile([128, TCH], f32, name=f"psk{tcj}_{p}", tag="mm")
            for kc in range(8):
                nc.tensor.matmul(
                    psk, wk_sb[kc][:, p * 128:(p + 1) * 128], xc[kc],
                    start=(kc == 0), stop=(kc == 7))
                if kc % 2 == 1:
                    yield
            kt = kcp.tile([128, TCH], bf16, name=f"k{tcj}_{p}", tag="kc")
            nc.scalar.activation(kt, psk, mybir.ActivationFunctionType.Identity,
                                 bias=bkt[:, p:p + 1])
            if dbg is not None and tcj == 0 and p == 0:
                nc.sync.dma_start(out=dbg["k"], in_=kt)
            k_sb[(p, tcj)] = kt
            yield
        for tt in range(4):
            ti = 4 * tcj + tt
            psv = ps_mm.tile([128, TCH], f32, name=f"psv{ti}", tag="mm")
            for kc in range(8):
                nc.tensor.matmul(
                    psv, xc[kc][:, tt * 128:(tt + 1) * 128], wv_sb[kc],
                    start=(kc == 0), stop=(kc == 7))
                if kc % 2 == 1:
                    yield
            vt = vp.tile([128, HL, D + 1], bf16, name=f"v{ti}", tag="v")
            nc.vector.tensor_add(
                vt[:, :, 0:D],
                psv.rearrange("p (h d) -> p h d", h=HL),
                bvb.rearrange("p (h d) -> p h d", h=HL))
            nc.vector.memset(vt[:, :, D:D + 1], 1.0)
            if dbg is not None and ti == 0:
                nc.sync.dma_start(out=dbg["v"], in_=vt)
            v_sb[ti] = vt
            yield

    def outproj_gen(j, yts):
        # the two co-chains of a token tile are emitted as a wave: both
        # run their fc0-2 matmuls before either fc3, so a late yts[3]
        # (last pair's normalize) doesn't block independent work in the
        # static PE stream
        for tt in range(4):
            row0 = j * TCH + tt * 128
            pso2 = []
            for co in range(2):
                pso = ps_mm.tile([128, TCH], f32,
                                 name=f"po{j}_{tt}_{co}", tag="mm")
                pso2.append(pso)
                for fc in range(3):
                    nc.tensor.matmul(
                        pso, yts[fc][:, tt * 128:(tt + 1) * 128],
                        wp_sb[fc][:, co * TCH:(co + 1) * TCH],
                        start=(fc == 0), stop=False)
                    if fc % 2 == 1:
                        yield
            for co in range(2):
                nc.tensor.matmul(
                    pso2[co], yts[3][:, tt * 128:(tt + 1) * 128],
                    wp_sb[3][:, co * TCH:(co + 1) * TCH],
                    start=False, stop=True)
                ost = ostp.tile([128, TCH], f32,
                                name=f"o{j}_{tt}_{co}", tag="o")
                nc.vector.tensor_copy(ost, pso2[co])
                nc.sync.dma_start(
                    out=out[row0:row0 + 128, co * TCH:(co + 1) * TCH],
                    in_=ost)
                yield

    def attention(j, fillq):
        def fill(n):
            while n > 0 and fillq:
                try:
                    next(fillq[0][2])
                    n -= 1
                except StopIteration:
                    fillq.pop(0)

        nkb = 4 * (j + 1)
        yts = []
        for p in range(NP):
            psyA = ps_y.tile([D + 1, TCH], f32, name=f"yA{j}_{p}", tag="y")
            psyB = ps_y.tile([D + 1, TCH], f32, name=f"yB{j}_{p}", tag="y")
            qA = q_sb[(p, j)][0:64, :]
            qB = q_sb[(p, j)][64:128, :]

            def emit_scores(kb):
                tck, wb = kb // 4, kb % 4
                kcc = k_sb[(p, tck)]
                pss = ps_s.tile([128, 2 * TCH], f32,
                                name=f"s{j}_{p}_{kb}", tag="s")
                so = max(0, 128 * (kb - 4 * j))  # valid q starts here
                nc.tensor.matmul(pss[:, so:TCH],
                                 kcc[0:64, wb * 128:(wb + 1) * 128],
                                 qA[:, so:TCH], start=True, stop=True)
                nc.tensor.matmul(pss[:, TCH + so:2 * TCH],
                                 kcc[64:128, wb * 128:(wb + 1) * 128],
                                 qB[:, so:TCH], start=True, stop=True)
                ee = ep.tile([128, 2 * TCH], bf16, name=f"e{j}_{p}_{kb}",
                             tag="e")
                di = kb - 4 * j
                if di <= 0:
                    nc.scalar.activation(ee, pss,
                                         mybir.ActivationFunctionType.Exp)
                else:
                    qo = 128 * di
                    evo = bass.AP(tensor=ee.tensor, offset=ee.offset + qo,
                                  ap=[ee.ap[0], [TCH, 2], [1, TCH - qo]])
                    evi = bass.AP(tensor=pss.tensor, offset=pss.offset + qo,
                                  ap=[pss.ap[0], [TCH, 2], [1, TCH - qo]])
                    nc.scalar.activation(evo, evi,
                                         mybir.ActivationFunctionType.Exp)
                if di >= 0:
                    qoff = 128 * di
                    evs = bass.AP(tensor=ee.tensor,
                                  offset=ee.offset + qoff,
                                  ap=[ee.ap[0], [TCH, 2], [1, 128]])
                    triv = bass.AP(tensor=tri.tensor, offset=tri.offset,
                                   ap=[tri.ap[0], [0, 2], [1, 128]])
                    nc.vector.tensor_mul(evs, evs, triv)
                if dbg is not None and j == 0 and p == 0 and kb < 2:
                    nc.sync.dma_start(out=dbg["e"][:, kb * TCH:(kb + 1) * TCH],
                                      in_=ee[:, 0:TCH])
                return ee

            def emit_av(kb, ee):
                di = kb - 4 * j
                qoff = 0 if di < 0 else 128 * di
                vt = v_sb[kb]
                nc.tensor.matmul(
                    psyA[:, qoff:TCH], vt[:, 2 * p, :],
                    ee[:, qoff:TCH],
                    start=(kb == 0), stop=(kb == nkb - 1))
                nc.tensor.matmul(
                    psyB[:, qoff:TCH], vt[:, 2 * p + 1, :],
                    ee[:, TCH + qoff:2 * TCH],
                    start=(kb == 0), stop=(kb == nkb - 1))

            pend = []
            for kb in range(nkb):
                ee = emit_scores(kb)
                if len(pend) >= 2:
                    emit_av(*pend.pop(0))
                fill(2)
                pend.append((kb, ee))
            for pr in pend:
                emit_av(*pr)

            yt = ytp.tile([128, TCH], bf16, name=f"yt{j}_{p}", tag="yt")
            for hh, psy in enumerate((psyA, psyB)):
                # free the PSUM bank fast: one whole-tile copy to SBUF, then
                # the whole normalize chain (DRAM-broadcast the denom row,
                # reciprocal, multiply) runs off SBUF with no PE and no
                # PSUM-slot involvement.
                ycp = rp.tile([D + 1, TCH], f32, name=f"yc{j}_{p}_{hh}",
                              tag="yc")
                nc.vector.tensor_copy(ycp, psy)
                dsc = drp.tile([1, TCH], f32, name=f"ds{j}_{p}_{hh}",
                               tag="dscr")
                nc.sync.dma_start(out=dsc, in_=ycp[D:D + 1, :])
                dbr = dbp.tile([64, TCH], f32, name=f"dbr{j}_{p}_{hh}",
                               tag="dbr")
                nc.sync.dma_start(out=dbr, in_=_bcast_part(dsc, 64))
                dbs = dbp.tile([64, TCH], f32, name=f"dbs{j}_{p}_{hh}",
                               tag="db")
                nc.vector.reciprocal_approx_fast(dbs, dbr)
                if dbg is not None and j == 0 and p == 0 and hh == 0:
                    nc.sync.dma_start(out=dbg["y"], in_=ycp)
                    nc.sync.dma_start(out=dbg["r"], in_=ycp[D:D + 1, :])
                    nc.sync.dma_start(out=dbg["db"], in_=dbs)
                if hh == 0:
                    nc.vector.tensor_mul(yt[0:64, :], ycp[0:D, :], dbs)
                else:
                    ytb = dbp.tile([64, TCH], bf16, name=f"ytb{j}_{p}",
                                   tag="ytb")
                    nc.vector.tensor_mul(ytb, ycp[0:D, :], dbs)
                    nc.sync.dma_start(out=yt[64:128, :], in_=ytb)
            if dbg is not None and j == 0 and p == 0:
                nc.sync.dma_start(out=dbg["yt"], in_=yt)
            yts.append(yt)
            fill(3)
        return yts

    # chunk 0 projections up front (q first while wk/wv still loading),
    # then attention(j) drip-feeds projections of chunk j+1 and the
    # output projection of chunk j-1 between its matmul bursts
    g0 = proj_gen(0, with_wq=True)
    next(g0)          # interleaved x + wq loads go first
    load_wkv()
    for _ in g0:
        pass
    load_wp()
    fillq = []   # entries: (kind, idx, generator)
    for j in range(NTC):
        if j + 1 < NTC:
            fillq.append(("proj", j + 1, proj_gen(j + 1)))
        # attention(j) requires proj(j) fully emitted: force-drain overdue
        for ent in [e for e in fillq if e[0] == "proj" and e[1] <= j]:
            for _ in ent[2]:
                pass
            fillq.remove(ent)
        yts = attention(j, fillq)
        fillq.append(("out", j, outproj_gen(j, yts)))
    for _, _, g in fillq:
        for _ in g:
            pass

    for p in reversed(ctxpools):
        p.release()


def _build(debug=False):
    key = ("nc", debug)
    if key in _CACHE:
        return _CACHE[key]
    nc = bacc.Bacc("TRN2", target_bir_lowering=False, debug=False)
    xT = nc.dram_tensor("xT", [NTC, 8, 128, TCH], bf16,
                    kind="ExternalInput").ap()
    wq = nc.dram_tensor("wqT", [C, F], bf16, kind="ExternalInput").ap()
    wk = nc.dram_tensor("wkT", [C, F], bf16, kind="ExternalInput").ap()
    wv = nc.dram_tensor("wvT", [C, F], bf16, kind="ExternalInput").ap()
    wp = nc.dram_tensor("wpT", [F, C], bf16, kind="ExternalInput").ap()
    bqs = nc.dram_tensor("bqs", [128, NP], f32, kind="ExternalInput").ap()
    bks = nc.dram_tensor("bks", [128, NP], f32, kind="ExternalInput").ap()
    bvr = nc.dram_tensor("bvr", [1, F], f32, kind="ExternalInput").ap()
    tri = nc.dram_tensor("tri", [128, 128], bf16, kind="ExternalInput").ap()
    out = nc.dram_tensor("out", [T, C], f32, kind="ExternalOutput").ap()
    dbg = None
    if debug:
        dbg = {
            "q": nc.dram_tensor("dbg_q", [128, TCH], bf16,
                                kind="ExternalOutput").ap(),
            "k": nc.dram_tensor("dbg_k", [128, TCH], bf16,
                                kind="ExternalOutput").ap(),
            "v": nc.dram_tensor("dbg_v", [128, HL, D + 1], bf16,
                                kind="ExternalOutput").ap(),
            "e": nc.dram_tensor("dbg_e", [128, 2 * TCH], bf16,
                                kind="ExternalOutput").ap(),
            "y": nc.dram_tensor("dbg_y", [D + 1, TCH], f32,
                                kind="ExternalOutput").ap(),
            "yt": nc.dram_tensor("dbg_yt", [128, TCH], bf16,
                                 kind="ExternalOutput").ap(),
            "r": nc.dram_tensor("dbg_r", [1, TCH], f32,
                                kind="ExternalOutput").ap(),
            "db": nc.dram_tensor("dbg_db", [64, TCH], f32,
                                 kind="ExternalOutput").ap(),
        }
    with tile.TileContext(nc) as tc:
        _emit(nc, tc, (xT, wq, wk, wv, wp, bqs, bks, bvr, tri, None, None,
                       out), dbg=dbg)
    nc.compile()
    _CACHE[key] = nc
    return nc


def make_in_maps(x, Wq, bq, Wk, bk, Wv, bv, Wp, bp):
    tri = np.triu(np.ones((128, 128), np.float32)).astype(ml_dtypes.bfloat16)
    in_maps = []
    for core in range(NCORES):
        b, g = core // G, core % G
        sl = slice(g * F, (g + 1) * F)
        in_maps.append({
            "xT": np.ascontiguousarray(
                x[b].T.reshape(8, 128, NTC, TCH).transpose(2, 0, 1, 3)
            ).astype(ml_dtypes.bfloat16),
            "wqT": np.ascontiguousarray(
                (Wq[sl, :] * 0.125).T).astype(ml_dtypes.bfloat16),
            "wkT": np.ascontiguousarray(Wk[sl, :].T).astype(
                ml_dtypes.bfloat16),
            "wvT": np.ascontiguousarray(Wv[sl, :].T).astype(
                ml_dtypes.bfloat16),
            "wpT": np.ascontiguousarray(Wp[:, sl].T).astype(
                ml_dtypes.bfloat16),
            "bqs": np.ascontiguousarray((bq[sl] * 0.125).reshape(NP, 128).T),
            "bks": np.ascontiguousarray(bk[sl].reshape(NP, 128).T),
            "bvr": bv[sl].reshape(1, F).copy(),
            "tri": tri,
        })
    return in_maps


def _maybe_install_trace_shim():
    """Enable NTFF profiling under axon when BASS_TRACE is set."""
    import sys
    import types
    if "antenv.axon_hooks" in sys.modules:
        return
    try:
        from trn_agent_boot.trn_boot import _ntff_profile_via_ctypes
        hook = _ntff_profile_via_ctypes("/opt/axon/libaxon_pjrt.so")
    except Exception:
        return
    mod = types.ModuleType("antenv.axon_hooks")
    mod.get_axon_ntff_profile_hook = lambda: hook
    mod.set_axon_ntff_profile_hook = lambda h: None
    sys.modules["antenv.axon_hooks"] = mod


def _run_device(x, Wq, bq, Wk, bk, Wv, bv, Wp, bp):
    global LAST_EXEC_TIME_NS
    trace = os.environ.get("BASS_TRACE", "") not in ("", "0")
    if trace:
        _maybe_install_trace_shim()
    nc = _build()
    in_maps = make_in_maps(x, Wq, bq, Wk, bk, Wv, bv, Wp, bp)
    res = run_bass_kernel_spmd(nc, in_maps, list(range(NCORES)), trace=trace)
    LAST_EXEC_TIME_NS = res.exec_time_ns
    out = np.empty((B, T, C), np.float32)
    for b in range(B):
        out[b] = res.results[2 * b]["out"] + res.results[2 * b + 1]["out"] + bp
    return out


def kernel(x, Wq, bq, Wk, bk, Wv, bv, Wp, bp):
    """Run on device in an isolated subprocess with retries: a rare
    transient device fault poisons the whole PJRT process, so isolation is
    the only way to recover and rerun."""
    global LAST_EXEC_TIME_NS
    import subprocess
    import sys
    import tempfile
    args = {k: np.asarray(v, np.float32) for k, v in dict(
        x=x, Wq=Wq, bq=bq, Wk=Wk, bk=bk, Wv=Wv, bv=bv, Wp=Wp, bp=bp).items()}
    if os.environ.get("BASS_KERNEL_INPROC"):
        return _run_device(**args)
    last = None
    for attempt in range(4):
        with tempfile.TemporaryDirectory() as td:
            inp = os.path.join(td, "in.npz")
            outp = os.path.join(td, "out.npz")
            np.savez(inp, **args)
            r = subprocess.run(
                [sys.executable, os.path.abspath(__file__),
                 "--worker", inp, outp],
                capture_output=True, text=True, timeout=3000)
            if r.returncode == 0 and os.path.exists(outp):
                with np.load(outp) as z:
                    out = z["out"]
                    t = z["exec_ns"]
                LAST_EXEC_TIME_NS = None if t < 0 else int(t)
                return out
            last = (r.returncode, r.stdout[-2000:], r.stderr[-2000:])
    raise RuntimeError(f"device run failed after retries: {last}")


def _worker(inp, outp):
    with np.load(inp) as z:
        args = {k: z[k] for k in z.files}
    out = _run_device(**args)
    t = -1 if LAST_EXEC_TIME_NS is None else int(LAST_EXEC_TIME_NS)
    np.savez(outp, out=out, exec_ns=t)


if __name__ == "__main__":
    import sys
    if len(sys.argv) == 4 and sys.argv[1] == "--worker":
        _worker(sys.argv[2], sys.argv[3])
